# revision 5
# baseline (speedup 1.0000x reference)
"""Trainium2 Bass kernel for nn_DecoderPolicyGradient (teacher-forced LSTM decoder).

Model: B=128, T=20, E=H=512, V=10000.
  xs[t] = features (t=0) | embed(captions[:, t-1])
  (h, c) = LSTM(xs[t], (h, c));  logits[t] = h @ W_lin.T + b_lin
  out = logits, time-major flattened: [T*B, V] fp32.

Sharding: pure data-parallel over batch, B/8 = 16 rows per NeuronCore, no
collectives. Per-core plan ("transposed": partition axis carries hidden/gate
dims, batch lives in the free dim):

  1. XgT[2048, 320] = W_ih @ xs.T + (b_ih + b_hh), computed in t-chunks:
     steps 0-4 up-front (gates step 0 on only 0.9 MB of input DMA), steps
     5-19 as three chunk-pieces emitted into the PE-idle tails of early
     recurrence steps. This both fills otherwise-dead gaps and keeps the
     PE HAM activity monitor warm (cold PE = 1.2 GHz = 2x slower N>=128
     matmuls) from phase 1 straight through the logits tail.
  2. 20 serial LSTM steps at B=16: gatesT[2048, 16] = W_hh @ h + XgT[:, t]
     as 16 m-tiles of [128, 16] (FWL LDWEIGHTS+MATMUL pairs ~25-27 ns,
     HAM-insensitive).
  3. logits[320, 10000] = H @ W_lin.T in bf16, staged into SBUF row-chunk
     buffers, written with few LARGE contiguous DMAs on the ACT HWDGE ring
     (inputs own the SP ring). Row chunks: [0,128) avail@step8 (full-width
     units), and three 64-row chunks avail@12/16/20 that pair vocab-half
     slices vertically in one psum tile (slice j in partitions 0-63,
     slice j+10 in 64-127) so no half-idle PE and copies stay
     partition-aligned. Units flow 3/step through EVERY step 8..19 so the
     HAM never drops before the tail.

Host side does data movement only: embedding row gather, weight re-layouts
(m-major contiguous quarters/halves so weight DMAs use 4-8KB descriptors),
final bf16->f32 upcast + 8 x [320, 10000] -> [2560, 10000] reassembly.
"""

import sys

sys.path.insert(0, "/opt/trn_rl_repo")

from contextlib import ExitStack

import ml_dtypes
import numpy as np

import concourse.mybir as mybir
import concourse.tile as tile
from concourse import bacc
from concourse.bass_utils import run_bass_kernel_spmd

BF16 = mybir.dt.bfloat16
F32 = mybir.dt.float32
AF = mybir.ActivationFunctionType

B, T, E, H, V = 128, 20, 512, 512, 10000
NC = 8
BL = B // NC  # 16 batch rows per core
R = BL * T  # 320 output rows per core
KT = 4  # k-tiles of 128 over E/H
GT = 16  # m-tiles of 128 over 4H
VS = 512  # vocab n-slice width
HV = 5120  # vocab half split for paired 64-row chunks
N_SLICES = [(s, min(VS, V - s)) for s in range(0, V, VS)]  # 20 slices
UNITS_PER_STEP = 3
XG_T0 = 5  # Xg t-chunks: steps [0, XG_T0) upfront, rest deferred into gaps

_cache = {}


def _build_nc(use_blin):
    nc = bacc.Bacc("TRN2", target_bir_lowering=False, debug=False)

    xsT_d = nc.dram_tensor("xsT", [128, KT, R], BF16, kind="ExternalInput").ap()
    wihT_d = nc.dram_tensor("wihT", [128, 4, KT, 512], BF16, kind="ExternalInput").ap()
    whhT_d = nc.dram_tensor("whhT", [128, 2, KT, 1024], BF16, kind="ExternalInput").ap()
    bsum_d = nc.dram_tensor("bsum", [128, GT], F32, kind="ExternalInput").ap()
    wlinT_d = nc.dram_tensor("wlinT", [128, KT, V], BF16, kind="ExternalInput").ap()
    blin_d = nc.dram_tensor("blin", [1, V], BF16, kind="ExternalInput").ap()
    h0T_d = nc.dram_tensor("h0T", [128, KT, BL], BF16, kind="ExternalInput").ap()
    c0T_d = nc.dram_tensor("c0T", [128, KT, BL], F32, kind="ExternalInput").ap()
    out_d = nc.dram_tensor("out", [R, V], BF16, kind="ExternalOutput").ap()

    with tile.TileContext(nc) as tc, ExitStack() as ctx:
        const = ctx.enter_context(tc.tile_pool(name="const", bufs=1))
        work = ctx.enter_context(tc.tile_pool(name="work", bufs=2))
        psum_g = ctx.enter_context(tc.tile_pool(name="psum_g", bufs=1, space="PSUM"))
        psum_l = ctx.enter_context(tc.tile_pool(name="psum_l", bufs=2, space="PSUM"))

        # ---- persistent SBUF tensors
        xsT = const.tile([128, KT, R], BF16)
        wihT = const.tile([128, 4, KT, 512], BF16)
        whhT = const.tile([128, 2, KT, 1024], BF16)
        bsum = const.tile([128, GT], F32)
        h0T = const.tile([128, KT, BL], BF16)
        c0T = const.tile([128, KT, BL], F32)
        blin = const.tile([1, V], BF16)
        ones = const.tile([1, 128], BF16)
        wlinT = const.tile([128, KT, V], BF16)
        xgT = const.tile([128, GT, R], BF16)
        hstore = const.tile([128, KT, R], BF16)
        obuf = [
            const.tile([128, V], BF16, name="obuf0", tag="obuf0"),
            const.tile([128, HV], BF16, name="obuf1", tag="obuf1"),
            const.tile([128, HV], BF16, name="obuf2", tag="obuf2"),
            const.tile([128, HV], BF16, name="obuf3", tag="obuf3"),
        ]

        # ---- input DMAs: one SP-HWDGE ring, FIFO = priority order.
        nc.sync.dma_start(xsT[:], xsT_d[:])
        nc.sync.dma_start(bsum[:], bsum_d[:])
        for q in range(4):
            nc.sync.dma_start(wihT[:, q], wihT_d[:, q])
        for hh in range(2):
            nc.sync.dma_start(whhT[:, hh], whhT_d[:, hh])
        nc.sync.dma_start(h0T[:], h0T_d[:])
        nc.sync.dma_start(c0T[:], c0T_d[:])
        if use_blin:
            nc.sync.dma_start(blin[:], blin_d[:])
            nc.gpsimd.memset(ones[:], 1.0)
        for s in range(0, V, 2560):
            w = min(2560, V - s)
            nc.sync.dma_start(wlinT[:, :, s : s + w], wlinT_d[:, :, s : s + w])

        # ---- Xg chunk emitter: XgT[:, :, t0*BL:t1*BL] for steps [t0, t1)
        def emit_xg(t0, t1, ms, me):
            lo, n = t0 * BL, (t1 - t0) * BL
            for m in range(ms, me):
                pxg = psum_l.tile([128, 160], F32, tag=f"pl{m % 2}")
                for k in range(KT):
                    nc.tensor.matmul(
                        pxg[:, :n],
                        wihT[:, m // 4, k, (m % 4) * 128 : (m % 4 + 1) * 128],
                        xsT[:, k, lo : lo + n],
                        start=(k == 0),
                        stop=(k == KT - 1),
                    )
                nc.scalar.activation(
                    xgT[:, m, lo : lo + n], pxg[:, :n], AF.Identity,
                    bias=bsum[:, m : m + 1],
                )

        # phase 1 upfront: steps [0, XG_T0) for all 16 m-tiles
        emit_xg(0, XG_T0, 0, GT)
        # deferred: steps [XG_T0, 20) in 6 pieces of 8 m-tiles, scheduled
        # into the tails of steps 1..6 (all done well before step XG_T0+5)
        xg_pieces = []
        for tc0 in range(XG_T0, T, 5):
            xg_pieces.append((tc0, tc0 + 5, 0, 8))
            xg_pieces.append((tc0, tc0 + 5, 8, 16))

        # ---- logits unit emitters (phase 3, interleaved into phase 2)
        copy_flip = [0]

        def mm_group(pl_ap, ms, rows, s, w):
            for k in range(KT):
                nc.tensor.matmul(
                    pl_ap,
                    hstore[:, k, ms : ms + rows],
                    wlinT[:, k, s : s + w],
                    start=(k == 0),
                    stop=(k == KT - 1) and not use_blin,
                )
            if use_blin:
                nc.tensor.matmul(
                    pl_ap, ones[:, :rows], blin[:, s : s + w],
                    start=False, stop=True,
                )

        def copy_out(dst_ap, src_ap):
            cp = nc.vector.tensor_copy if copy_flip[0] % 2 == 0 else nc.scalar.copy
            copy_flip[0] += 1
            cp(dst_ap, src_ap)

        def emit_unit(ci, j):
            if ci == 0:
                s, w = N_SLICES[j]
                pl = psum_l.tile([128, VS], F32, tag=f"pl{j % 2}")
                mm_group(pl[:, :w], 0, 128, s, w)
                copy_out(obuf[0][:, s : s + w], pl[:, :w])
            else:
                # 64-row chunk: rows 128+64*(ci-1); vertical pairing: psum
                # top half = vocab slice j, bottom half = slice j+10.
                ms = 128 + 64 * (ci - 1)
                sl, wl = N_SLICES[j]
                sh, wh = N_SLICES[j + 10]
                pl = psum_l.tile([128, VS], F32, tag=f"pl{j % 2}")
                mm_group(pl[0:64, :wl], ms, 64, sl, wl)
                mm_group(pl[64:128, :wh], ms, 64, sh, wh)
                copy_out(obuf[ci][:, j * VS : j * VS + VS], pl[:])

        def fire_dma(ci, half):
            if ci == 0:
                s0, s1 = (0, HV) if half == 0 else (HV, V)
                nc.scalar.dma_start(out_d[0:128, s0:s1], obuf[0][:, s0:s1])
            else:
                ms = 128 + 64 * (ci - 1)
                nc.scalar.dma_start(out_d[ms : ms + 64, 0:HV], obuf[ci][0:64, :])
                nc.scalar.dma_start(
                    out_d[ms : ms + 64, HV:V], obuf[ci][64:128, 0 : V - HV]
                )

        queue = (
            [(0, j, 8) for j in range(20)]
            + [(1, j, 12) for j in range(10)]
            + [(2, j, 16) for j in range(10)]
            + [(3, j, 20) for j in range(10)]
        )
        done = {0: 0, 1: 0, 2: 0, 3: 0}

        def emit_and_track(ci, j):
            emit_unit(ci, j)
            done[ci] += 1
            if ci == 0:
                if done[0] == 10:
                    fire_dma(0, 0)
                elif done[0] == 20:
                    fire_dma(0, 1)
            elif done[ci] == 10:
                fire_dma(ci, 0)

        # ---- phase 2: 20 serial LSTM steps (B = 16)
        cT_prev = c0T
        hT_prev = h0T
        qi = 0
        for t in range(T):
            lo = t * BL
            pg = []
            for g in range(4):
                p = psum_g.tile([128, 4, BL], F32, tag=f"pg{g}")
                pg.append(p)
                for mi in range(4):
                    m = g * 4 + mi
                    for k in range(KT):
                        nc.tensor.matmul(
                            p[:, mi, :],
                            whhT[:, m // 8, k, (m % 8) * 128 : (m % 8 + 1) * 128],
                            hT_prev[:, k, :],
                            start=(k == 0),
                            stop=(k == KT - 1),
                        )
            gates = work.tile([128, GT, BL], BF16, tag="gates")
            for g in range(4):
                nc.vector.tensor_add(
                    gates[:, 4 * g : 4 * g + 4, :],
                    pg[g][:],
                    xgT[:, 4 * g : 4 * g + 4, lo : lo + BL],
                )
            act_if = work.tile([128, 8, BL], BF16, tag="actif")
            act_g = work.tile([128, 4, BL], BF16, tag="actg")
            act_o = work.tile([128, 4, BL], BF16, tag="acto")
            nc.scalar.activation(act_if[:], gates[:, 0:8, :], AF.Sigmoid)
            nc.scalar.activation(act_g[:], gates[:, 8:12, :], AF.Tanh)
            nc.scalar.activation(act_o[:], gates[:, 12:16, :], AF.Sigmoid)
            ig = work.tile([128, 4, BL], F32, tag="ig")
            fc = work.tile([128, 4, BL], F32, tag="fc")
            nc.vector.tensor_mul(ig[:], act_if[:, 0:4, :], act_g[:])
            nc.vector.tensor_mul(fc[:], act_if[:, 4:8, :], cT_prev[:])
            c_new = work.tile([128, 4, BL], F32, tag="c")
            nc.vector.tensor_add(c_new[:], fc[:], ig[:])
            tc_b = work.tile([128, 4, BL], BF16, tag="tanhc")
            nc.scalar.activation(tc_b[:], c_new[:], AF.Tanh)
            h_new = hstore[:, :, lo : lo + BL]
            nc.vector.tensor_mul(h_new, act_o[:], tc_b[:])
            cT_prev = c_new
            hT_prev = h_new
            # gap fill: deferred Xg pieces first (steps 1..len(pieces)),
            # then logits units 3/step from step 8
            if 1 <= t <= len(xg_pieces):
                emit_xg(*xg_pieces[t - 1])
            n_emit = 0
            while (
                qi < len(queue)
                and n_emit < UNITS_PER_STEP
                and queue[qi][2] <= t
            ):
                emit_and_track(queue[qi][0], queue[qi][1])
                qi += 1
                n_emit += 1

        while qi < len(queue):
            emit_and_track(queue[qi][0], queue[qi][1])
            qi += 1

    nc.compile()
    return nc


def _prep_inputs(features, captions, h0, c0, embed_w, W_ih, W_hh, b_ih, b_hh,
                 W_lin, b_lin):
    """Host-side layout prep (data movement only). Returns per-core in_maps."""
    bf = ml_dtypes.bfloat16
    f32 = np.float32

    features = np.asarray(features, f32)
    captions = np.asarray(captions)
    h0 = np.asarray(h0, f32)
    c0 = np.asarray(c0, f32)
    embed_w = np.asarray(embed_w, f32)
    W_ih = np.asarray(W_ih, f32)
    W_hh = np.asarray(W_hh, f32)
    b_ih = np.asarray(b_ih, f32)
    b_hh = np.asarray(b_hh, f32)
    W_lin = np.asarray(W_lin, f32)
    b_lin = np.asarray(b_lin, f32)

    # xs: [B, T, E] = [features, embed(captions[:, :T-1])]
    xs = np.empty((B, T, E), f32)
    xs[:, 0, :] = features
    xs[:, 1:, :] = embed_w[captions[:, : T - 1]]

    def to_kpm(w):  # [512, M] -> [128, KT, M] with row = k*128 + p
        return np.ascontiguousarray(w.reshape(KT, 128, w.shape[1]).transpose(1, 0, 2))

    def to_chunks(w_kpm, n):  # [128, KT, 2048] -> [128, n, KT, 2048/n] m-major
        return np.ascontiguousarray(
            w_kpm.reshape(128, KT, n, 2048 // n).transpose(0, 2, 1, 3)
        )

    wihT = to_chunks(to_kpm(W_ih.T), 4).astype(bf)
    whhT = to_chunks(to_kpm(W_hh.T), 2).astype(bf)
    wlinT = to_kpm(W_lin.T).astype(bf)
    bsum = np.ascontiguousarray((b_ih + b_hh).reshape(GT, 128).T).astype(f32)
    blin = b_lin.reshape(1, V).astype(bf)

    in_maps = []
    for j in range(NC):
        sl = slice(j * BL, (j + 1) * BL)
        x = xs[sl]  # [BL, T, E]
        xsT = x.transpose(2, 1, 0).reshape(KT, 128, T * BL).transpose(1, 0, 2)
        h0T = h0[sl].T.reshape(KT, 128, BL).transpose(1, 0, 2)
        c0T = c0[sl].T.reshape(KT, 128, BL).transpose(1, 0, 2)
        in_maps.append(
            {
                "xsT": np.ascontiguousarray(xsT).astype(bf),
                "wihT": wihT,
                "whhT": whhT,
                "bsum": bsum,
                "wlinT": wlinT,
                "blin": blin,
                "h0T": np.ascontiguousarray(h0T).astype(bf),
                "c0T": np.ascontiguousarray(c0T).astype(f32),
            }
        )
    return in_maps


def kernel(**inputs) -> np.ndarray:
    maxlen = int(inputs.get("maxlen", T))
    assert maxlen == T, f"kernel hardcodes T={T}, got maxlen={maxlen}"
    use_blin = bool(np.any(np.asarray(inputs["b_lin"])))
    key = ("nc", use_blin)
    if key not in _cache:
        _cache[key] = _build_nc(use_blin)
    nc = _cache[key]
    in_maps = _prep_inputs(
        inputs["features"], inputs["captions"], inputs["h0"], inputs["c0"],
        inputs["embed_w"], inputs["W_ih"], inputs["W_hh"], inputs["b_ih"],
        inputs["b_hh"], inputs["W_lin"], inputs["b_lin"],
    )
    res = run_bass_kernel_spmd(nc, in_maps, list(range(NC)))
    # reassemble: core j rows (t*BL + b) -> full rows (t*B + j*BL + b)
    out = np.empty((T * B, V), np.float32)
    ov = out.reshape(T, NC, BL, V)
    for j in range(NC):
        ov[:, j] = res.results[j]["out"].reshape(T, BL, V).astype(np.float32)
    return out


# revision 6
# speedup vs baseline: 1.1578x; 1.1578x over previous
"""Trainium2 Bass kernel for nn_DecoderPolicyGradient (teacher-forced LSTM decoder).

Model: B=128, T=20, E=H=512, V=10000.
  xs[t] = features (t=0) | embed(captions[:, t-1])
  (h, c) = LSTM(xs[t], (h, c));  logits[t] = h @ W_lin.T + b_lin
  out = logits, time-major flattened: [T*B, V] fp32.

Sharding: pure data-parallel over batch, B/8 = 16 rows per NeuronCore, no
collectives. Per-core plan ("transposed": partition axis carries hidden/gate
dims, batch lives in the free dim):

  1. XgT[2048, 320] = W_ih @ xs.T + (b_ih + b_hh): one batched matmul over
     all 20 steps (N=320 amortizes LDWEIGHTS); bias rides the ACT psum->SBUF
     copy.
  2. 20 serial LSTM steps at B=16: gatesT[2048, 16] = W_hh @ h + XgT[:, t]
     as 16 m-tiles of [128, 16]. W_hh is fp8e4m3 (numerics validated:
     rel_l2 8.7e-3 vs 2e-2 budget) which doubles the FWL weight-load rate
     - the recurrence matmul stream is LDWEIGHTS-bound.
  3. logits[320, 10000] = H @ W_lin.T in bf16, staged into SBUF row-chunk
     buffers, written with few LARGE contiguous DMAs on the ACT HWDGE ring
     (inputs own the SP ring). Row chunks: the one unavoidable 64-row chunk
     (320 = 2*128 + 64) goes FIRST (rows 0-63, ready after step 4) so its
     half-idle-PE units fill early-step gaps that nothing else can use, and
     they keep the PE HAM activity monitor warm (cold PE = 1.2 GHz = 2x
     slower N=512 matmuls). It pairs vocab slices vertically in one psum
     tile (slice j in partitions 0-63, j+10 in 64-127) so copies stay
     partition-aligned. Then rows 64-191 (ready@12) and rows 192-319
     (tail) as full-width units.

Host side does data movement only: embedding row gather, weight re-layouts
(m-major contiguous quarters/halves so weight DMAs use 4-8KB descriptors),
final bf16->f32 upcast + 8 x [320, 10000] -> [2560, 10000] reassembly.
"""

import sys

sys.path.insert(0, "/opt/trn_rl_repo")

from contextlib import ExitStack

import ml_dtypes
import numpy as np

import concourse.mybir as mybir
import concourse.tile as tile
from concourse import bacc
from concourse.bass_utils import run_bass_kernel_spmd

BF16 = mybir.dt.bfloat16
FP8 = mybir.dt.float8e4
F32 = mybir.dt.float32
AF = mybir.ActivationFunctionType

B, T, E, H, V = 128, 20, 512, 512, 10000
NC = 8
BL = B // NC  # 16 batch rows per core
R = BL * T  # 320 output rows per core
KT = 4  # k-tiles of 128 over E/H
GT = 16  # m-tiles of 128 over 4H
VS = 512  # vocab n-slice width
HV = 5120  # vocab half split
N_SLICES = [(s, min(VS, V - s)) for s in range(0, V, VS)]  # 20 slices
# per-step logits-unit allowance: ramp up as cheap units become available
ALLOW = {t: (1 if t < 8 else 2 if t < 12 else 3) for t in range(T)}

_cache = {}


def _build_nc(use_blin):
    nc = bacc.Bacc("TRN2", target_bir_lowering=False, debug=False)

    xsT_d = nc.dram_tensor("xsT", [128, KT, R], BF16, kind="ExternalInput").ap()
    wihT_d = nc.dram_tensor("wihT", [128, 4, KT, 512], BF16, kind="ExternalInput").ap()
    whhT_d = nc.dram_tensor("whhT", [128, 2, KT, 1024], FP8, kind="ExternalInput").ap()
    bsum_d = nc.dram_tensor("bsum", [128, GT], F32, kind="ExternalInput").ap()
    wlinT_d = nc.dram_tensor("wlinT", [128, KT, V], BF16, kind="ExternalInput").ap()
    blin_d = nc.dram_tensor("blin", [1, V], BF16, kind="ExternalInput").ap()
    h0T_d = nc.dram_tensor("h0T", [128, KT, BL], BF16, kind="ExternalInput").ap()
    c0T_d = nc.dram_tensor("c0T", [128, KT, BL], F32, kind="ExternalInput").ap()
    out_d = nc.dram_tensor("out", [R, V], BF16, kind="ExternalOutput").ap()

    with tile.TileContext(nc) as tc, ExitStack() as ctx:
        const = ctx.enter_context(tc.tile_pool(name="const", bufs=1))
        work = ctx.enter_context(tc.tile_pool(name="work", bufs=2))
        psum_g = ctx.enter_context(tc.tile_pool(name="psum_g", bufs=1, space="PSUM"))
        psum_l = ctx.enter_context(tc.tile_pool(name="psum_l", bufs=2, space="PSUM"))

        # ---- persistent SBUF tensors
        xsT = const.tile([128, KT, R], BF16)
        wihT = const.tile([128, 4, KT, 512], BF16)
        whhT = const.tile([128, 2, KT, 1024], FP8)
        bsum = const.tile([128, GT], F32)
        h0T = const.tile([128, KT, BL], BF16)
        c0T = const.tile([128, KT, BL], F32)
        blin = const.tile([1, V], BF16)
        ones = const.tile([1, 128], BF16)
        wlinT = const.tile([128, KT, V], BF16)
        xgT = const.tile([128, GT, R], BF16)
        hstore = const.tile([128, KT, R], BF16)
        obuf = [
            const.tile([128, HV], BF16, name="obuf0", tag="obuf0"),
            const.tile([128, V], BF16, name="obuf1", tag="obuf1"),
            const.tile([128, V], BF16, name="obuf2", tag="obuf2"),
        ]

        # ---- input DMAs: one SP-HWDGE ring, FIFO = priority order.
        nc.sync.dma_start(xsT[:], xsT_d[:])
        nc.sync.dma_start(bsum[:], bsum_d[:])
        for q in range(4):
            nc.sync.dma_start(wihT[:, q], wihT_d[:, q])
        for hh in range(2):
            nc.sync.dma_start(whhT[:, hh], whhT_d[:, hh])
        nc.sync.dma_start(h0T[:], h0T_d[:])
        nc.sync.dma_start(c0T[:], c0T_d[:])
        if use_blin:
            nc.sync.dma_start(blin[:], blin_d[:])
            nc.gpsimd.memset(ones[:], 1.0)
        for s in range(0, V, 2560):
            w = min(2560, V - s)
            nc.sync.dma_start(wlinT[:, :, s : s + w], wlinT_d[:, :, s : s + w])

        # ---- phase 1: XgT[2048, R] = W_ih @ xs.T + bsum
        for m in range(GT):
            pxg = psum_l.tile([128, R], F32, tag=f"pl{m % 2}")
            for k in range(KT):
                nc.tensor.matmul(
                    pxg[:],
                    wihT[:, m // 4, k, (m % 4) * 128 : (m % 4 + 1) * 128],
                    xsT[:, k, :],
                    start=(k == 0),
                    stop=(k == KT - 1),
                )
            nc.scalar.activation(
                xgT[:, m, :], pxg[:], AF.Identity, bias=bsum[:, m : m + 1]
            )

        # ---- logits unit emitters (phase 3, interleaved into phase 2)
        copy_flip = [0]

        def mm_group(pl_ap, ms, rows, s, w):
            for k in range(KT):
                nc.tensor.matmul(
                    pl_ap,
                    hstore[:, k, ms : ms + rows],
                    wlinT[:, k, s : s + w],
                    start=(k == 0),
                    stop=(k == KT - 1) and not use_blin,
                )
            if use_blin:
                nc.tensor.matmul(
                    pl_ap, ones[:, :rows], blin[:, s : s + w],
                    start=False, stop=True,
                )

        def copy_out(dst_ap, src_ap):
            cp = nc.vector.tensor_copy if copy_flip[0] % 2 == 0 else nc.scalar.copy
            copy_flip[0] += 1
            cp(dst_ap, src_ap)

        def emit_unit(ci, j):
            if ci == 0:
                # 64-row chunk rows 0-63: vertical pairing (slice j top,
                # slice j+10 bottom)
                sl, wl = N_SLICES[j]
                sh, wh = N_SLICES[j + 10]
                pl = psum_l.tile([128, VS], F32, tag=f"pl{j % 2}")
                mm_group(pl[0:64, :wl], 0, 64, sl, wl)
                mm_group(pl[64:128, :wh], 0, 64, sh, wh)
                copy_out(obuf[0][:, j * VS : j * VS + VS], pl[:])
            else:
                ms = 64 if ci == 1 else 192
                s, w = N_SLICES[j]
                pl = psum_l.tile([128, VS], F32, tag=f"pl{j % 2}")
                mm_group(pl[:, :w], ms, 128, s, w)
                copy_out(obuf[ci][:, s : s + w], pl[:, :w])

        def fire_dma(ci, half):
            if ci == 0:
                nc.scalar.dma_start(out_d[0:64, 0:HV], obuf[0][0:64, :])
                nc.scalar.dma_start(out_d[0:64, HV:V], obuf[0][64:128, 0 : V - HV])
            else:
                ms = 64 if ci == 1 else 192
                s0, s1 = (0, HV) if half == 0 else (HV, V)
                nc.scalar.dma_start(
                    out_d[ms : ms + 128, s0:s1], obuf[ci][:, s0:s1]
                )

        queue = (
            [(0, j, 4) for j in range(10)]
            + [(1, j, 12) for j in range(20)]
            + [(2, j, 20) for j in range(20)]
        )
        done = {0: 0, 1: 0, 2: 0}

        def emit_and_track(ci, j):
            emit_unit(ci, j)
            done[ci] += 1
            if ci == 0:
                if done[0] == 10:
                    fire_dma(0, 0)
            else:
                if done[ci] == 10:
                    fire_dma(ci, 0)
                elif done[ci] == 20:
                    fire_dma(ci, 1)

        # ---- phase 2: 20 serial LSTM steps (B = 16)
        cT_prev = c0T
        hT_prev = h0T
        qi = 0
        for t in range(T):
            lo = t * BL
            pg = []
            for g in range(4):
                p = psum_g.tile([128, 4, BL], F32, tag=f"pg{g}")
                pg.append(p)
                for mi in range(4):
                    m = g * 4 + mi
                    for k in range(KT):
                        nc.tensor.matmul(
                            p[:, mi, :],
                            whhT[:, m // 8, k, (m % 8) * 128 : (m % 8 + 1) * 128],
                            hT_prev[:, k, :],
                            start=(k == 0),
                            stop=(k == KT - 1),
                        )
            gates = work.tile([128, GT, BL], BF16, tag="gates")
            for g in range(4):
                nc.vector.tensor_add(
                    gates[:, 4 * g : 4 * g + 4, :],
                    pg[g][:],
                    xgT[:, 4 * g : 4 * g + 4, lo : lo + BL],
                )
            act_if = work.tile([128, 8, BL], BF16, tag="actif")
            act_g = work.tile([128, 4, BL], BF16, tag="actg")
            act_o = work.tile([128, 4, BL], BF16, tag="acto")
            nc.scalar.activation(act_if[:], gates[:, 0:8, :], AF.Sigmoid)
            nc.scalar.activation(act_g[:], gates[:, 8:12, :], AF.Tanh)
            nc.scalar.activation(act_o[:], gates[:, 12:16, :], AF.Sigmoid)
            ig = work.tile([128, 4, BL], F32, tag="ig")
            fc = work.tile([128, 4, BL], F32, tag="fc")
            nc.vector.tensor_mul(ig[:], act_if[:, 0:4, :], act_g[:])
            nc.vector.tensor_mul(fc[:], act_if[:, 4:8, :], cT_prev[:])
            c_new = work.tile([128, 4, BL], F32, tag="c")
            nc.vector.tensor_add(c_new[:], fc[:], ig[:])
            tc_b = work.tile([128, 4, BL], BF16, tag="tanhc")
            nc.scalar.activation(tc_b[:], c_new[:], AF.Tanh)
            h_new = hstore[:, :, lo : lo + BL]
            nc.vector.tensor_mul(h_new, act_o[:], tc_b[:])
            cT_prev = c_new
            hT_prev = h_new
            n_emit = 0
            while (
                qi < len(queue)
                and n_emit < ALLOW[t]
                and queue[qi][2] <= t
            ):
                emit_and_track(queue[qi][0], queue[qi][1])
                qi += 1
                n_emit += 1

        while qi < len(queue):
            emit_and_track(queue[qi][0], queue[qi][1])
            qi += 1

    nc.compile()
    return nc


def _prep_inputs(features, captions, h0, c0, embed_w, W_ih, W_hh, b_ih, b_hh,
                 W_lin, b_lin):
    """Host-side layout prep (data movement only). Returns per-core in_maps."""
    bf = ml_dtypes.bfloat16
    f8 = ml_dtypes.float8_e4m3
    f32 = np.float32

    features = np.asarray(features, f32)
    captions = np.asarray(captions)
    h0 = np.asarray(h0, f32)
    c0 = np.asarray(c0, f32)
    embed_w = np.asarray(embed_w, f32)
    W_ih = np.asarray(W_ih, f32)
    W_hh = np.asarray(W_hh, f32)
    b_ih = np.asarray(b_ih, f32)
    b_hh = np.asarray(b_hh, f32)
    W_lin = np.asarray(W_lin, f32)
    b_lin = np.asarray(b_lin, f32)

    # xs: [B, T, E] = [features, embed(captions[:, :T-1])]
    xs = np.empty((B, T, E), f32)
    xs[:, 0, :] = features
    xs[:, 1:, :] = embed_w[captions[:, : T - 1]]

    def to_kpm(w):  # [512, M] -> [128, KT, M] with row = k*128 + p
        return np.ascontiguousarray(w.reshape(KT, 128, w.shape[1]).transpose(1, 0, 2))

    def to_chunks(w_kpm, n):  # [128, KT, 2048] -> [128, n, KT, 2048/n] m-major
        return np.ascontiguousarray(
            w_kpm.reshape(128, KT, n, 2048 // n).transpose(0, 2, 1, 3)
        )

    wihT = to_chunks(to_kpm(W_ih.T), 4).astype(bf)
    whhT = to_chunks(to_kpm(W_hh.T), 2).astype(f8)
    wlinT = to_kpm(W_lin.T).astype(bf)
    bsum = np.ascontiguousarray((b_ih + b_hh).reshape(GT, 128).T).astype(f32)
    blin = b_lin.reshape(1, V).astype(bf)

    in_maps = []
    for j in range(NC):
        sl = slice(j * BL, (j + 1) * BL)
        x = xs[sl]  # [BL, T, E]
        xsT = x.transpose(2, 1, 0).reshape(KT, 128, T * BL).transpose(1, 0, 2)
        h0T = h0[sl].T.reshape(KT, 128, BL).transpose(1, 0, 2)
        c0T = c0[sl].T.reshape(KT, 128, BL).transpose(1, 0, 2)
        in_maps.append(
            {
                "xsT": np.ascontiguousarray(xsT).astype(bf),
                "wihT": wihT,
                "whhT": whhT,
                "bsum": bsum,
                "wlinT": wlinT,
                "blin": blin,
                "h0T": np.ascontiguousarray(h0T).astype(bf),
                "c0T": np.ascontiguousarray(c0T).astype(f32),
            }
        )
    return in_maps


def kernel(**inputs) -> np.ndarray:
    maxlen = int(inputs.get("maxlen", T))
    assert maxlen == T, f"kernel hardcodes T={T}, got maxlen={maxlen}"
    use_blin = bool(np.any(np.asarray(inputs["b_lin"])))
    key = ("nc", use_blin)
    if key not in _cache:
        _cache[key] = _build_nc(use_blin)
    nc = _cache[key]
    in_maps = _prep_inputs(
        inputs["features"], inputs["captions"], inputs["h0"], inputs["c0"],
        inputs["embed_w"], inputs["W_ih"], inputs["W_hh"], inputs["b_ih"],
        inputs["b_hh"], inputs["W_lin"], inputs["b_lin"],
    )
    res = run_bass_kernel_spmd(nc, in_maps, list(range(NC)))
    # reassemble: core j rows (t*BL + b) -> full rows (t*B + j*BL + b)
    out = np.empty((T * B, V), np.float32)
    ov = out.reshape(T, NC, BL, V)
    for j in range(NC):
        ov[:, j] = res.results[j]["out"].reshape(T, BL, V).astype(np.float32)
    return out
